# revision 13
# baseline (speedup 1.0000x reference)
"""MixtureOfBidders Trainium2 kernel: 8-core expert-parallel on intermediate dim.

Sharding: each core owns an I-slice (896 of 7168) of base FFN weights and all
per-expert LoRA B/A factors restricted to that slice. Every core sees all
T=2048 tokens, computes routing (fp32), gate/up = base + lora in PSUM (f32r
matmuls with identity-replay of the base), hid = silu(g)*u, weighted z and
rank-space partials, then its partial down-projection output [H, T].
A ReduceScatter per token-block sums partials across cores; host assembles.
"""
import sys
sys.path.insert(0, '/opt/trn_rl_repo')
import numpy as np

import concourse.bacc as bacc
import concourse.mybir as mybir
import concourse.tile as tile
from concourse.bass_utils import run_bass_kernel_spmd

F32 = mybir.dt.float32
F32R = mybir.dt.float32r
ALU = mybir.AluOpType
ACTF = mybir.ActivationFunctionType
AX = mybir.AxisListType

NCORES = 8
T = 2048          # tokens (B*S)
H = 2048          # hidden
I = 7168          # intermediate
IC = I // NCORES  # 896 per core
E = 8             # experts
R = 64            # lora rank
ER = E * R        # 512
SCALING = 16.0 / 64.0
TB = 512          # token block
NBLK = T // TB    # 4
NH = H // 128     # 16 h-chunks
NIC = IC // 128   # 7 i-chunks per core
NHC = H // 128    # 16 out h-chunks
NM = ER // 128    # 4 er-chunks

_CACHE = {}


def _build():
    nc = bacc.Bacc("TRN2", target_bir_lowering=False, debug=False,
                   num_devices=NCORES)
    dram = {}
    def inp(name, shape, dt=F32R):
        dram[name] = nc.dram_tensor(name, list(shape), dt, kind="ExternalInput")
        return dram[name]

    xT = inp("xT", (H, T))
    confw = inp("confw", (H, 8), F32)
    confb = inp("confb", (1, 8), F32)
    wealth = inp("wealth", (1, 8), F32)
    ident = inp("ident", (128, 128))          # f32r identity
    sel = inp("sel", (8, E * 128))            # one-hot row selectors
    wg = inp("wg", (NIC, H, 128))
    wu = inp("wu", (NIC, H, 128))
    dt_w = inp("dt", (NHC, IC, 128))
    ga = inp("ga", (NM, H, 128))
    ua = inp("ua", (NM, H, 128))
    gb = inp("gb", (NM, 128, IC))   # expert pairs stacked on partitions
    ub = inp("ub", (NM, 128, IC))
    da = inp("da", (E, NIC, 128, R))
    db = inp("db", (NHC, R, E, 128))
    out_ext = nc.dram_tensor("out", [T // NCORES, T], F32, kind="ExternalOutput")

    with tile.TileContext(nc) as tc:
        with tc.tile_pool(name="const", bufs=1) as cpool, \
             tc.tile_pool(name="sb", bufs=1) as sb, \
             tc.tile_pool(name="ps", bufs=1, space="PSUM") as ps, \
             tc.tile_pool(name="dpool", bufs=1, space="DRAM") as dpool:

            # ---- constants ----
            ident_t = cpool.tile([128, 128], F32R)
            nc.sync.dma_start(ident_t[:], ident[:])
            sel_t = cpool.tile([8, E * 128], F32R)
            nc.sync.dma_start(sel_t[:], sel[:])
            confw_t = cpool.tile([128, 128], F32)
            nc.sync.dma_start(
                confw_t.rearrange("p (hc e) -> p hc e", hc=NH),
                confw.rearrange("(hc p) e -> p hc e", p=128))
            confb_sm = cpool.tile([1, 8], F32)
            nc.sync.dma_start(confb_sm[:], confb[:])
            wealth_sm = cpool.tile([1, 8], F32)
            nc.sync.dma_start(wealth_sm[:], wealth[:])
            confb_bc = cpool.tile([128, 8], F32)
            nc.gpsimd.partition_broadcast(confb_bc[:], confb_sm[:])
            wealth_bc = cpool.tile([128, 8], F32)
            nc.gpsimd.partition_broadcast(wealth_bc[:], wealth_sm[:])

            for blk in range(NBLK):
                tsl = slice(blk * TB, (blk + 1) * TB)

                # ---- load x block ----
                xt = []
                for h in range(NH):
                    x_t = sb.tile([128, TB], F32R, name=f"x{blk}_{h}",
                                  tag="xt", bufs=17)
                    nc.sync.dma_start(x_t[:], xT[h * 128:(h + 1) * 128, tsl])
                    xt.append(x_t)

                # ---- phase A: conf + routing ----
                wT = sb.tile([8, TB], F32R, name=f"wT{blk}", tag="wT", bufs=2)
                for tt in range(TB // 128):
                    ps_c = ps.tile([128, 8], F32, name=f"psc{blk}_{tt}",
                                   tag="conf", bufs=1)
                    for h in range(NH):
                        nc.tensor.matmul(
                            ps_c[:],
                            xt[h].bitcast(F32)[:, tt * 128:(tt + 1) * 128],
                            confw_t.rearrange("p (hc e) -> p hc e", hc=NH)[:, h, :],
                            start=(h == 0), stop=(h == NH - 1))
                    logits = sb.tile([128, 8], F32, tag="rt8", bufs=4)
                    nc.vector.tensor_tensor(logits[:], ps_c[:], confb_bc[:],
                                            op=ALU.add)
                    conf = sb.tile([128, 8], F32, tag="rt8", bufs=4)
                    nc.scalar.activation(conf[:], logits[:], ACTF.Sigmoid)
                    bids = sb.tile([128, 8], F32, tag="rt8", bufs=4)
                    nc.vector.tensor_tensor(bids[:], conf[:], wealth_bc[:],
                                            op=ALU.mult)
                    m1 = sb.tile([128, 1], F32, tag="rt1", bufs=8)
                    nc.vector.reduce_max(m1[:], bids[:], axis=AX.X)
                    mask1 = sb.tile([128, 8], F32, tag="rt8", bufs=4)
                    nc.vector.tensor_scalar(mask1[:], bids[:], m1[:], None,
                                            op0=ALU.is_equal)
                    masked = sb.tile([128, 8], F32, tag="rt8", bufs=4)
                    nc.vector.scalar_tensor_tensor(
                        masked[:], mask1[:], -1e30, bids[:],
                        op0=ALU.mult, op1=ALU.add)
                    m2 = sb.tile([128, 1], F32, tag="rt1", bufs=8)
                    nc.vector.reduce_max(m2[:], masked[:], axis=AX.X)
                    mask2 = sb.tile([128, 8], F32, tag="rt8", bufs=4)
                    nc.vector.tensor_scalar(mask2[:], bids[:], m2[:], None,
                                            op0=ALU.is_equal)
                    d = sb.tile([128, 1], F32, tag="rt1", bufs=8)
                    nc.vector.tensor_scalar(d[:], m2[:], m1[:], None,
                                            op0=ALU.subtract)
                    ed = sb.tile([128, 1], F32, tag="rt1", bufs=8)
                    nc.scalar.activation(ed[:], d[:], ACTF.Exp)
                    den = sb.tile([128, 1], F32, tag="rt1", bufs=8)
                    nc.vector.tensor_scalar(den[:], ed[:], 1.0, None,
                                            op0=ALU.add)
                    rec = sb.tile([128, 1], F32, tag="rt1", bufs=8)
                    nc.vector.reciprocal(rec[:], den[:])
                    s2 = sb.tile([128, 1], F32, tag="rt1", bufs=8)
                    nc.vector.tensor_tensor(s2[:], ed[:], rec[:], op=ALU.mult)
                    w1p = sb.tile([128, 8], F32, tag="rt8", bufs=4)
                    nc.vector.tensor_scalar(w1p[:], mask1[:], rec[:], None,
                                            op0=ALU.mult)
                    wfin = sb.tile([128, 8], F32, tag="rt8", bufs=4)
                    nc.vector.scalar_tensor_tensor(
                        wfin[:], mask2[:], s2[:], w1p[:],
                        op0=ALU.mult, op1=ALU.add)
                    ps_wt = ps.tile([8, 128], F32, name=f"pswt{blk}_{tt}",
                                    tag="wtp", bufs=1)
                    nc.tensor.transpose(ps_wt[:], wfin[:],
                                        ident_t.bitcast(F32)[:])
                    nc.scalar.copy(wT[:, tt * 128:(tt + 1) * 128], ps_wt[:])

                # ---- phase B: tR = loraA @ x ----
                trg, tru = [], []
                for gu, (src, dst) in enumerate(((ga, trg), (ua, tru))):
                    for m in range(NM):
                        ga_t = sb.tile([128, H], F32R, name=f"ga{blk}_{gu}_{m}",
                                       tag="wst", bufs=4)
                        nc.sync.dma_start(
                            ga_t.rearrange("p (hc j) -> p hc j", hc=NH),
                            src[m].rearrange("(hc p) j -> p hc j", p=128))
                        ps_t = ps.tile([128, TB], F32, name=f"pstr{blk}_{gu}_{m}",
                                       tag="mmA", bufs=4)
                        gav = ga_t.rearrange("p (hc j) -> p hc j", hc=NH)
                        for h in range(NH):
                            nc.tensor.matmul(ps_t[:], gav[:, h, :], xt[h][:],
                                             start=(h == 0), stop=(h == NH - 1))
                        tr_t = sb.tile([128, TB], F32R, name=f"tr{blk}_{gu}_{m}",
                                       tag="tr", bufs=8)
                        nc.scalar.copy(tr_t[:], ps_t[:])
                        dst.append(tr_t)

                # ---- phase C: base gate/up ----
                bg, bu = [], []
                for gu, (src, dst) in enumerate(((wg, bg), (wu, bu))):
                    for ic in range(NIC):
                        w_t = sb.tile([128, H], F32R, name=f"w{blk}_{gu}_{ic}",
                                      tag="wst", bufs=4)
                        nc.sync.dma_start(
                            w_t.rearrange("p (hc i) -> p hc i", hc=NH),
                            src[ic].rearrange("(hc p) i -> p hc i", p=128))
                        ps_t = ps.tile([128, TB], F32, name=f"psb{blk}_{gu}_{ic}",
                                       tag="mmA", bufs=4)
                        wv = w_t.rearrange("p (hc i) -> p hc i", hc=NH)
                        for h in range(NH):
                            nc.tensor.matmul(ps_t[:], wv[:, h, :], xt[h][:],
                                             start=(h == 0), stop=(h == NH - 1))
                        b_t = sb.tile([128, TB], F32R, name=f"b{blk}_{gu}_{ic}",
                                      tag="base", bufs=14)
                        nc.scalar.copy(b_t[:], ps_t[:])
                        dst.append(b_t)

                # ---- phase D: experts ----
                z = [sb.tile([128, TB], F32R, name=f"z{blk}_{ic}", tag="z",
                             bufs=7) for ic in range(NIC)]
                pw = []
                gbp, ubp = {}, {}
                for e in range(E):
                    ps_w = ps.tile([128, TB], F32, name=f"psw{blk}_{e}",
                                   tag="mmA", bufs=4)
                    nc.tensor.matmul(ps_w[:], sel_t[:, e * 128:(e + 1) * 128],
                                     wT[:], start=True, stop=True)
                    wbc = sb.tile([128, TB], F32, name=f"wbc{blk}_{e}",
                                  tag="wbc", bufs=2)
                    nc.scalar.copy(wbc[:], ps_w[:])
                    m, half = e // 2, (e % 2) * 64
                    if e % 2 == 0:
                        gbp[m] = sb.tile([128, IC], F32R, name=f"gb{blk}_{m}",
                                         tag="gbt", bufs=4)
                        nc.sync.dma_start(gbp[m][:], gb[m])
                        ubp[m] = sb.tile([128, IC], F32R, name=f"ub{blk}_{m}",
                                         tag="gbt", bufs=4)
                        nc.sync.dma_start(ubp[m][:], ub[m])
                    gb_t = gbp[m][half:half + 64, :]
                    ub_t = ubp[m][half:half + 64, :]
                    da_t = sb.tile([128, NIC * R], F32R, name=f"da{blk}_{e}",
                                   tag="dat", bufs=2)
                    nc.sync.dma_start(
                        da_t.rearrange("p (ic r) -> p ic r", ic=NIC),
                        da[e].rearrange("ic p r -> p ic r"))
                    dav = da_t.rearrange("p (ic r) -> p ic r", ic=NIC)
                    trg_e = trg[m][half:half + 64, :]
                    tru_e = tru[m][half:half + 64, :]
                    ps_p = ps.tile([64, TB], F32, name=f"psp{blk}_{e}",
                                   tag="pp", bufs=2)
                    for ic in range(NIC):
                        ps_g = ps.tile([128, TB], F32, name=f"psg{blk}_{e}_{ic}",
                                       tag="mmA", bufs=4)
                        nc.tensor.matmul(ps_g[:], ident_t[:], bg[ic][:],
                                         start=True, stop=False)
                        nc.tensor.matmul(ps_g[:],
                                         gb_t[:, ic * 128:(ic + 1) * 128],
                                         trg_e, start=False, stop=True)
                        ps_u = ps.tile([128, TB], F32, name=f"psu{blk}_{e}_{ic}",
                                       tag="mmA", bufs=4)
                        nc.tensor.matmul(ps_u[:], ident_t[:], bu[ic][:],
                                         start=True, stop=False)
                        nc.tensor.matmul(ps_u[:],
                                         ub_t[:, ic * 128:(ic + 1) * 128],
                                         tru_e, start=False, stop=True)
                        sg = sb.tile([128, TB], F32, name=f"sg{blk}_{e}_{ic}",
                                     tag="sg", bufs=2)
                        nc.scalar.activation(sg[:], ps_g[:], ACTF.Silu)
                        hid = sb.tile([128, TB], F32R, name=f"hid{blk}_{e}_{ic}",
                                      tag="hid", bufs=2)
                        nc.vector.tensor_tensor(hid[:], sg[:], ps_u[:],
                                                op=ALU.mult)
                        if e == 0:
                            nc.vector.tensor_tensor(z[ic][:], hid[:], wbc[:],
                                                    op=ALU.mult)
                        else:
                            hw = sb.tile([128, TB], F32,
                                         name=f"hw{blk}_{e}_{ic}",
                                         tag="hw", bufs=2)
                            nc.vector.tensor_tensor(hw[:], hid[:], wbc[:],
                                                    op=ALU.mult)
                            nc.vector.tensor_tensor(z[ic][:], z[ic][:], hw[:],
                                                    op=ALU.add)
                        nc.tensor.matmul(ps_p[:], dav[:, ic, :], hid[:],
                                         start=(ic == 0), stop=(ic == NIC - 1))
                    pw_t = sb.tile([64, TB], F32R, name=f"pw{blk}_{e}",
                                   tag="pw", bufs=8)
                    nc.vector.tensor_tensor(pw_t[:], ps_p[:], wbc[0:64, :],
                                            op=ALU.mult)
                    pw.append(pw_t)

                # ---- phase E: down ----
                outT_blk = dpool.tile([H, TB], F32, name=f"outT{blk}")
                for hc in range(NHC):
                    dt_t = sb.tile([128, NIC * 128], F32R,
                                   name=f"dtw{blk}_{hc}", tag="dtw", bufs=3)
                    nc.sync.dma_start(
                        dt_t.rearrange("p (ic h) -> p ic h", ic=NIC),
                        dt_w[hc].rearrange("(ic p) h -> p ic h", p=128))
                    dtv = dt_t.rearrange("p (ic h) -> p ic h", ic=NIC)
                    db_t = sb.tile([64, E * 128], F32R, name=f"dbw{blk}_{hc}",
                                   tag="dbw", bufs=2)
                    nc.sync.dma_start(
                        db_t.rearrange("p (e h) -> p e h", e=E), db[hc])
                    dbv = db_t.rearrange("p (e h) -> p e h", e=E)
                    ps_o = ps.tile([128, TB], F32, name=f"pso{blk}_{hc}",
                                   tag="mmA", bufs=4)
                    for ic in range(NIC):
                        nc.tensor.matmul(ps_o[:], dtv[:, ic, :], z[ic][:],
                                         start=(ic == 0), stop=False)
                    for e in range(E):
                        nc.tensor.matmul(ps_o[:], dbv[:, e, :], pw[e][:],
                                         start=False, stop=(e == E - 1))
                    o_sb = sb.tile([128, TB], F32, name=f"o{blk}_{hc}",
                                   tag="osb", bufs=3)
                    nc.scalar.copy(o_sb[:], ps_o[:])
                    nc.sync.dma_start(outT_blk[hc * 128:(hc + 1) * 128, :],
                                      o_sb[:])

                rs_blk = dpool.tile([H // NCORES, TB], F32, name=f"rs{blk}")
                nc.gpsimd.collective_compute(
                    "ReduceScatter", ALU.add,
                    replica_groups=[list(range(NCORES))],
                    ins=[outT_blk.opt()], outs=[rs_blk.opt()])
                nc.sync.dma_start(out_ext[:, tsl], rs_blk[:])

    nc.compile()
    return nc


def _prep(inputs):
    """Host-side sharding/layout. Returns in_maps (8 dicts of np arrays)."""
    hs = np.asarray(inputs["hidden_states"], np.float32)
    x = np.ascontiguousarray(hs.reshape(-1, H).T)            # [H, T]
    confw = np.ascontiguousarray(np.asarray(inputs["conf_w"], np.float32).T)
    confb = np.asarray(inputs["conf_b"], np.float32).reshape(1, 8)
    wealth = np.asarray(inputs["wealth"], np.float32).reshape(1, 8)
    ident = np.eye(128, dtype=np.float32)
    sel = np.kron(np.eye(8, dtype=np.float32),
                  np.ones((1, 128), np.float32))             # [8, 1024]
    gA = np.asarray(inputs["gA"], np.float32)
    uA = np.asarray(inputs["uA"], np.float32)
    gB = np.asarray(inputs["gB"], np.float32)
    uB = np.asarray(inputs["uB"], np.float32)
    dA = np.asarray(inputs["dA"], np.float32)
    dB = np.asarray(inputs["dB"], np.float32)
    wg_f = np.asarray(inputs["base_gate_w"], np.float32)
    wu_f = np.asarray(inputs["base_up_w"], np.float32)
    wd_f = np.asarray(inputs["base_down_w"], np.float32)

    def lhsT_blocks(w2d, nblk):   # [K, M_total] -> [nblk, K, 128]
        K = w2d.shape[0]
        return np.ascontiguousarray(
            w2d.reshape(K, nblk, 128).transpose(1, 0, 2))

    gaT = lhsT_blocks(gA.reshape(ER, H).T, NM)               # [4, H, 128]
    uaT = lhsT_blocks(uA.reshape(ER, H).T, NM)
    in_maps = []
    for c in range(NCORES):
        sl = slice(c * IC, (c + 1) * IC)
        wgT = lhsT_blocks(wg_f[sl].T, NIC)                   # [7, H, 128]
        wuT = lhsT_blocks(wu_f[sl].T, NIC)
        X = np.ascontiguousarray(wd_f[:, sl].T)              # [IC, H]
        dtw = np.ascontiguousarray(
            X.reshape(IC, NHC, 128).transpose(1, 0, 2))      # [16, IC, 128]
        gbT = np.ascontiguousarray(
            (gB[:, sl, :] * SCALING).transpose(0, 2, 1).reshape(NM, 128, IC))
        ubT = np.ascontiguousarray(
            (uB[:, sl, :] * SCALING).transpose(0, 2, 1).reshape(NM, 128, IC))
        daT = np.ascontiguousarray(
            dA[:, :, sl].transpose(0, 2, 1).reshape(E, NIC, 128, R))
        dbT = np.ascontiguousarray(
            (dB * SCALING).reshape(E, NHC, 128, R).transpose(1, 3, 0, 2))
        in_maps.append({
            "xT": x, "confw": confw, "confb": confb, "wealth": wealth,
            "ident": ident, "sel": sel, "wg": wgT, "wu": wuT, "dt": dtw,
            "ga": gaT, "ua": uaT, "gb": gbT, "ub": ubT, "da": daT, "db": dbT,
        })
    return in_maps


def kernel(**inputs):
    if "nc" not in _CACHE:
        _CACHE["nc"] = _build()
    nc = _CACHE["nc"]
    in_maps = _prep(inputs)
    res = run_bass_kernel_spmd(nc, in_maps, core_ids=list(range(NCORES)),
                               **_CACHE.get("run_kwargs", {}))
    _CACHE["last_result"] = res
    outT = np.concatenate([res.results[c]["out"] for c in range(NCORES)],
                          axis=0)                            # [H, T]
    B, S = 2, 1024
    return np.ascontiguousarray(outT.T).reshape(B, S, H).astype(np.float32)
